# revision 14
# baseline (speedup 1.0000x reference)
"""MoE layer (top-2 of 8 experts, H=1024, FFN=4096) on 8 TRN2 NeuronCores.

Expert-parallel: core e holds expert e's weights resident in SBUF. The
(tiny) router runs on host; tokens are gathered per-expert into
capacity-padded batches, each core runs the expert FFN and the host
applies the gate weight and scatter-adds the two expert contributions.

Device layout per core (C = per-expert token capacity, multiple of 8),
tokens processed in PSUM-sized chunks of <=256:
  GEMM1  h[f, c] = w1s[h, f].T @ x[h, c]    (F on partitions, tokens free)
  GEMM2  y[h, c] = w2s[f, h].T @ h[f, c]    (H on partitions, tokens free)
Both GEMMs stream the token axis, so compute scales with C exactly (no
128-ceil on token tiles). y accumulates in PSUM across all 32 F-tiles
(8 slabs x 4): 8 accumulators packed 2-per-bank; since start=True on a
matmul clears the whole PSUM bank, only the first matmul per BANK uses
start=True — the bank-mate's first matmul relies on the per-element
has_written bits to overwrite. Eviction is one ScalarE Identity per
H-tile folding b2 in as a per-partition bias; gate + top-2 combine on
host.

Weights/x are staged in DRAM pre-swizzled to match SBUF layout exactly
(slab-major, partition rows contiguous) so weight DMA moves in 2KB+
packets; v2 measured 256B packets on w1 and starved the PE for ~15us.

GEMMs run in bf16 (fp32 matmul is 4x slower; fp8 DoubleRow fails the
2e-2 accuracy gate: each quantized tensor alone contributes ~2.7e-2).
"""

import os

os.environ.setdefault("NEURON_RT_RESET_CORES", "1")

import ml_dtypes
import numpy as np

import concourse.bass as bass  # noqa: F401  (bass types via bacc)
import concourse.mybir as mybir
from concourse import bacc
from concourse.tile import TileContext
from concourse.bass_utils import run_bass_kernel_spmd

H = 1024
E = 8
F = 4096
TOPK = 2
P = 128
N_CORES = 8
FP32 = mybir.dt.float32
FP16 = mybir.dt.float16
BF16 = mybir.dt.bfloat16

NTH = 8            # F slabs
FT = F // NTH      # 512 F columns per slab
MF = FT // P       # 4 f-tiles of 128 per slab
KH = H // P        # 8 contraction tiles for GEMM1
HT = H // P        # 8 output H-tiles for GEMM2

# PE warmup matmuls (128 rows each): sized so the warmup ends about when
# the first real GEMM1 group's DMA deps land (~9.5us).
N_WARMUP = 10

_cache: dict = {}

# Test-harness knobs (harness-safe defaults): set TRACE=True before calling
# kernel() to profile the device run; exec time lands in LAST_EXEC_TIME_NS.
TRACE = False
LAST_EXEC_TIME_NS = None


def _chunks(C: int):
    """Near-even token chunks, multiples of 8, each <=512. (A small
    final chunk to shorten the drain was tried and lost more to per-chunk
    gelu/issue overheads than it saved.)"""
    nch = -(-C // 512)
    u = C // 8
    units = [u // nch + (1 if i < u % nch else 0) for i in range(nch)]
    widths = [un * 8 for un in units]
    assert sum(widths) == C and all(0 < w <= 512 for w in widths)
    out = []
    off = 0
    for w in widths:
        out.append((off, w))
        off += w
    return out


def _build(C: int):
    """Build + compile the per-core expert-FFN program for capacity C."""
    assert C % 8 == 0
    cbs = _chunks(C)

    nc = bacc.Bacc("TRN2", target_bir_lowering=False, debug=False,
                   num_devices=N_CORES)

    # All big inputs pre-swizzled on host to the exact SBUF layout:
    # w1s row th*128+p, col m*1024 + k*128 + f2 = w1.T[k*128+p, th*512+m*128+f2]
    # w2s row th*128+p, col m*1024+h  = w2.T[(th*4+m)*128+p, h]
    # xc  row p,        col 8*coff + k*ck + c = x[coff+c, k*128+p]
    # per-slab concatenation of w1 and w2 so one DMA trigger (~600ns of SP
    # issue time each, flat regardless of size) loads a whole slab
    WS1 = KH * FT                  # 4096 w1 cols per slab
    WSL = WS1 + MF * H             # + 4096 w2 cols
    wall = nc.dram_tensor("wall", [NTH * P, WSL], BF16, kind="ExternalInput")
    xc = nc.dram_tensor("xc", [P, KH * C], BF16, kind="ExternalInput")
    b1c = nc.dram_tensor("b1c", [P, F // P], FP32, kind="ExternalInput")
    out = nc.dram_tensor("out", [H, C], FP16, kind="ExternalOutput")

    out_v = out.rearrange("(t p) c -> p t c", p=P)   # [128, 8, C]

    GELU = mybir.ActivationFunctionType.Gelu

    with TileContext(nc) as tc:
        with (
            tc.tile_pool(name="const", bufs=1) as constp,
            tc.tile_pool(name="xp", bufs=1) as xp,
            tc.tile_pool(name="wp", bufs=1) as wp,
            tc.tile_pool(name="hp", bufs=1) as hp,
            tc.tile_pool(name="op", bufs=2) as op,
            tc.tile_pool(name="ps1", bufs=2, space="PSUM") as ps1p,
            tc.tile_pool(name="psy", bufs=1, space="PSUM") as psyp,
        ):
            # PE warmup: the NEFF prologue keeps the DMA queue dead until
            # ~8.2us and the first GEMM1 group's deps (x0 first half +
            # slab-0 m0 w1) land ~8.5us. A short run of dummy matmuls
            # starts the p-state ramp without overrunning that point (the
            # v3 72-matmul warmup held the PE until 17us while deps were
            # ready at 10.6us). PE executes in order, so warmup length
            # directly delays the first real matmul.
            zt = constp.tile([P, 2 * P], BF16, tag="zt")
            nc.vector.memset(zt[:], 0.0)
            wups = psyp.tile([P, 2, 256], FP32, tag="warm")
            for i in range(N_WARMUP):
                # rotate targets so the Tile WAW chain is 2 matmuls deep
                nc.tensor.matmul(wups[:, i % 2, :128], zt[:, :P],
                                 zt[:, :128], start=True, stop=True)

            x_t = []

            def alloc_x(ci):
                coff, ck = cbs[ci]
                t = xp.tile([P, KH, ck], BF16, tag=f"x{ci}", name=f"x{ci}")
                x_t.append(t)
                return t, coff, ck

            # Two hardware DGE queues on TRN2: SP and Activation. Weights
            # ride the SP queue; x chunks + b1 ride the Act queue (idle
            # for DMA at the critical start window), so the first GEMM1
            # group's deps (x0 first half || slab-0 m0 w1) transfer
            # CONCURRENTLY instead of serially on one queue. Per-queue
            # emission order = arrival order = consumption order.
            x0, coff0, ck0 = alloc_x(0)
            nc.scalar.dma_start(out=x0[:, :KH // 2, :],
                                in_=xc[:, :KH // 2 * ck0])
            w_sb = []
            for th in range(NTH):
                t = wp.tile([P, WSL], BF16, tag=f"w_{th}", name=f"w_{th}")
                w_sb.append(t)
            nc.sync.dma_start(out=w_sb[0][:, :KH * P],
                              in_=wall[:P, :KH * P])
            b1_sb = constp.tile([P, F // P], FP32, tag="b1")
            nc.scalar.dma_start(out=x0[:, KH // 2:, :],
                                in_=xc[:, KH // 2 * ck0:KH * ck0])
            nc.scalar.dma_start(out=b1_sb[:], in_=b1c[:])
            for m in range(1, MF):
                nc.sync.dma_start(
                    out=w_sb[0][:, m * KH * P:(m + 1) * KH * P],
                    in_=wall[:P, m * KH * P:(m + 1) * KH * P])
            for th in range(1, NTH):
                rows = wall[th * P:(th + 1) * P, :]
                nc.sync.dma_start(out=w_sb[th][:, :WS1], in_=rows[:, :WS1])
            for th in range(NTH):
                rows = wall[th * P:(th + 1) * P, :]
                nc.sync.dma_start(out=w_sb[th][:, WS1:], in_=rows[:, WS1:])

            def w1sl(th, m, k):
                return w_sb[th][:, m * KH * P + k * P:m * KH * P + (k + 1) * P]

            def w2sl(th, m, ht):
                base = WS1 + m * H + ht * P
                return w_sb[th][:, base:base + P]

            for ci in range(1, len(cbs)):
                t, coff, ck = alloc_x(ci)
                nc.scalar.dma_start(
                    out=t[:], in_=xc[:, KH * coff:KH * (coff + ck)])

            for ci, (coff, ck) in enumerate(cbs):
                last_chunk = ci == len(cbs) - 1

                # 4 full-bank y accumulators, reused by the two GEMM2
                # half-H passes
                def ytile(q, half):
                    return psyp.tile([P, 512], FP32, tag=f"y{q}",
                                     name=f"y{q}_{ci}_{half}")

                # evictions go to DVE (ScalarE is the gelu critical path)
                # into a staging tile; one store DMA per half-H pass.
                # fp16 staging: 2x DVE throughput on the evict and half the
                # store bytes (b2 is folded in on the host, not here).
                def o4tile(half):
                    return op.tile([P, 4, 512], FP16, tag="o4",
                                   name=f"o4_{ci}_{half}")

                # GEMM1 phase: h for all 8 slabs staged in SBUF
                hL = hp.tile([P, NTH, MF, 512], BF16, tag="h",
                             name=f"h_{ci}")
                for th in range(NTH):
                    for m in range(MF):
                        pt = ps1p.tile([P, 512], FP32, tag="ps1")
                        for k in range(KH):
                            nc.tensor.matmul(
                                pt[:, :ck],
                                w1sl(th, m, k),
                                x_t[ci][:, k, :],
                                start=(k == 0), stop=(k == KH - 1),
                            )
                        nc.scalar.activation(
                            hL[:, th, m, :ck], pt[:, :ck], GELU,
                            bias=b1_sb[:, th * MF + m:th * MF + m + 1],
                        )

                # GEMM2: two passes of 4 H-tiles. th outer inside a pass
                # so the w2 slabs stream at the DMA-sustainable rate on
                # chunk 0; on the final chunk's second pass, ht outer so
                # each accumulator closes early and evictions + stores
                # chase the remaining matmuls
                for half in range(2):
                    y_q = [ytile(q, half) for q in range(4)]
                    o4 = o4tile(half)

                    def evict(q):
                        nc.vector.tensor_copy(
                            o4[:, q, :ck], y_q[q][:, :ck])

                    if last_chunk and half == 1:
                        # per-ht stores: a single merged store at the very
                        # end exposes its whole ~350KB transfer
                        for q in range(4):
                            ht = 4 * half + q
                            for th in range(NTH):
                                for m in range(MF):
                                    nc.tensor.matmul(
                                        y_q[q][:, :ck],
                                        w2sl(th, m, ht),
                                        hL[:, th, m, :ck],
                                        start=(th == 0 and m == 0),
                                        stop=(th == NTH - 1 and m == MF - 1),
                                    )
                            evict(q)
                            nc.scalar.dma_start(
                                out=out_v[:, ht:ht + 1, coff:coff + ck],
                                in_=o4[:, q:q + 1, :ck])
                        continue
                    else:
                        for th in range(NTH):
                            for m in range(MF):
                                for q in range(4):
                                    nc.tensor.matmul(
                                        y_q[q][:, :ck],
                                        w2sl(th, m, 4 * half + q),
                                        hL[:, th, m, :ck],
                                        start=(th == 0 and m == 0),
                                        stop=(th == NTH - 1 and m == MF - 1),
                                    )
                        for q in range(4):
                            evict(q)
                    nc.scalar.dma_start(
                        out=out_v[:, 4 * half:4 * half + 4, coff:coff + ck],
                        in_=o4[:, :, :ck])

    nc.compile()
    return nc


def _route(x: np.ndarray, router_w: np.ndarray):
    """Host router: top-2 expert ids + softmax gates per token."""
    logits = x @ router_w.T                                   # [T, E]
    top_i = np.argsort(-logits, axis=1, kind="stable")[:, :TOPK]
    top_v = np.take_along_axis(logits, top_i, axis=1)
    mx = top_v.max(axis=1, keepdims=True)
    ex = np.exp(top_v - mx)
    rw = ex / ex.sum(axis=1, keepdims=True)
    return top_i, rw.astype(np.float32)


def _swizzle_wall(w1e: np.ndarray, w2e: np.ndarray) -> np.ndarray:
    # w1 [F, H] -> rows th*128+p, cols m*1024 + k*128 + f2
    a = w1e.reshape(NTH, MF, P, KH, P).transpose(0, 4, 1, 3, 2).reshape(
        NTH * P, KH * FT)
    # w2 [H, F] -> w2.T [F, H] -> rows th*128+p, cols m*1024+h
    b = w2e.T.reshape(NTH, MF, P, H).transpose(0, 2, 1, 3).reshape(
        NTH * P, MF * H)
    return np.ascontiguousarray(
        np.concatenate([a, b], axis=1)).astype(ml_dtypes.bfloat16)


def kernel(hidden_states, router_w, w1, b1, w2, b2):
    hidden_states = np.ascontiguousarray(np.asarray(hidden_states, np.float32))
    router_w = np.ascontiguousarray(np.asarray(router_w, np.float32))
    w1 = np.asarray(w1, np.float32)
    b1 = np.asarray(b1, np.float32)
    w2 = np.asarray(w2, np.float32)
    b2 = np.asarray(b2, np.float32)

    B, S, _ = hidden_states.shape
    T = B * S
    x = hidden_states.reshape(T, H)

    top_i, rw = _route(x, router_w)

    sel_idx = []
    sel_gate = []
    for e in range(E):
        mask = top_i == e                                     # [T, K]
        rows = np.nonzero(mask.any(axis=1))[0]
        g = rw[rows[:, None], np.argmax(mask[rows], axis=1)[:, None]][:, 0]
        sel_idx.append(rows)
        sel_gate.append(g.astype(np.float32))

    # One job per (expert, token-chunk). Normally each expert fits in one
    # chunk and a single 8-core SPMD round runs everything; with an extreme
    # routing skew an expert's batch is split into <=C_MAX chunks (bounded
    # by SBUF) and extra rounds run.
    C_MAX = 2048
    jobs = []                                   # (expert, rows, gates)
    for e in range(E):
        rows, g = sel_idx[e], sel_gate[e]
        for off in range(0, max(len(rows), 1), C_MAX):
            jobs.append((e, rows[off:off + C_MAX], g[off:off + C_MAX]))

    n_rounds = -(-len(jobs) // N_CORES)
    cmax = max(len(r) for _, r, _ in jobs)
    C = max(P, -(-cmax // 8) * 8)

    if C not in _cache:
        _cache[C] = _build(C)
    nc = _cache[C]
    cbs = _chunks(C)

    w_bf = {}
    def expert_inputs(e):
        if e not in w_bf:
            w_bf[e] = {
                "wall": _swizzle_wall(w1[e], w2[e]),
                "b1c": np.ascontiguousarray(b1[e].reshape(F // P, P).T),
            }
        return w_bf[e]

    global LAST_EXEC_TIME_NS
    LAST_EXEC_TIME_NS = 0
    out = np.zeros((T, H), np.float32)
    for r in range(n_rounds):
        batch = jobs[r * N_CORES:(r + 1) * N_CORES]
        while len(batch) < N_CORES:
            batch.append((0, sel_idx[0][:0], sel_gate[0][:0]))
        in_maps = []
        for e, rows, g in batch:
            n_e = len(rows)
            xpad = np.zeros((C, H), np.float32)
            xpad[:n_e] = x[rows]
            # chunk-major swizzle: col 8*coff + k*ck + c = x[coff+c, k*128+p]
            xc = np.concatenate(
                [xpad[coff:coff + ck].reshape(ck, KH, P)
                 .transpose(2, 1, 0).reshape(P, KH * ck)
                 for coff, ck in cbs], axis=1)
            in_maps.append({
                "xc": np.ascontiguousarray(xc).astype(ml_dtypes.bfloat16),
                **expert_inputs(e),
            })

        res = run_bass_kernel_spmd(nc, in_maps, list(range(N_CORES)), trace=TRACE)
        if res.exec_time_ns:
            LAST_EXEC_TIME_NS += res.exec_time_ns

        for core, (e, rows, g) in enumerate(batch):
            if len(rows):
                # y comes back [H, C] fp16 WITHOUT b2; host folds b2 into
                # the gate + top-2 combine (row indices unique per job)
                y = res.results[core]["out"][:, :len(rows)].T.astype(np.float32)
                out[rows] += g[:, None] * (y + b2[e][None, :])
    return out.reshape(B, S, H)



# revision 18
# speedup vs baseline: 1.0412x; 1.0412x over previous
"""MoE layer (top-2 of 8, H=1024, FFN=4096) on 8 TRN2 NeuronCores —
4-way expert-F-split for load balance.

Two quads of 4 cores; quad q serves 4 experts (experts interleaved by
sorted token count so rank-r segment sizes match across quads). Core j
of a quad holds F-rows [j*1024, (j+1)*1024) of ALL 4 of its experts
(16.8MB, same weight footprint as expert-parallel). Each core streams
the quad's full token set once per segment (4 segments, one per
expert); per-core rows = 128 * sum(S_s) with S_s = rank-r max over the
two quads. vs expert-parallel's 512*maxcount: always <=, and ~3% less
at this seed's near-balanced routing (4128 vs 4256 token-equivalents).
Host sums the 4 F-slice partials (fp16) per expert and folds b2 + the
top-2 softmax gates.

Device schedule per core: segments sequential; per segment, token
chunks of <=512 (PSUM width). Per chunk: GEMM1 (2 slabs x 4 f-tiles x
8 k) -> gelu+b1 -> GEMM2 (8 ht x 8 accum) -> fp16 evict on DVE ->
store. x tiles rotate through 4 SBUF slots [P,8,512]; host pads each
chunk to the 512 slot so DMA runs are contiguous. x triggers for chunk
g+3 are emitted after chunk g's GEMM1 so the SP engine never blocks on
a slot-free wait. DMA emission order = arrival order = consumption
order; the first GEMM1 group's deps (x chunk-0 + slab-0 m0 w1) are
interleaved in ~131-262KB pieces so the first real matmul starts
~9.8us (NEFF prologue keeps DMA dead until ~8.3us; a short PE warmup
covers the p-state ramp until then).

GEMMs in bf16 (fp32 4x slower; fp8 fails the 2e-2 gate per v2
measurements). Weights pre-swizzled on host to exact SBUF layout.
"""

import os

os.environ.setdefault("NEURON_RT_RESET_CORES", "1")

import ml_dtypes
import numpy as np

import concourse.bass as bass  # noqa: F401  (bass types via bacc)
import concourse.mybir as mybir
from concourse import bacc
from concourse.tile import TileContext
from concourse.bass_utils import run_bass_kernel_spmd

H = 1024
E = 8
F = 4096
TOPK = 2
P = 128
N_CORES = 8
FP32 = mybir.dt.float32
FP16 = mybir.dt.float16
BF16 = mybir.dt.bfloat16

NSEG = 4           # experts per quad / segments per core
NSPLIT = 4         # cores sharing each expert's F dim
FL = F // NSPLIT   # 1024 F rows per core per expert
NTHL = FL // 512   # 2 slabs (of 512 F) per expert per core
MF = 512 // P      # 4 f-tiles per slab
KH = H // P        # 8 contraction tiles for GEMM1
HT = H // P        # 8 output H-tiles for GEMM2
NSL = NSEG * NTHL  # 8 slabs total per core
CKS = 512          # x slot width (PSUM-limited chunk cap)

WS1 = KH * 512               # 4096 w1 cols per slab
WSL = WS1 + MF * H           # + 4096 w2 cols

# PE warmup matmuls (128 rows each): cover the p-state ramp until the
# first real GEMM1 group's DMA deps land (~10us).
N_WARMUP = 12

_cache: dict = {}

TRACE = False
LAST_EXEC_TIME_NS = None


def _chunks(C: int):
    """Near-even token chunks, multiples of 8, each <=512."""
    nch = -(-C // CKS)
    u = C // 8
    units = [u // nch + (1 if i < u % nch else 0) for i in range(nch)]
    widths = [un * 8 for un in units]
    assert sum(widths) == C and all(0 < w <= CKS for w in widths)
    out = []
    off = 0
    for w in widths:
        out.append((off, w))
        off += w
    return out


def _build(segs: tuple):
    """Per-core program: NSEG sequential expert segments of sizes segs."""
    assert all(s % 8 == 0 and s > 0 for s in segs)
    seg_cbs = [_chunks(s) for s in segs]
    seg_off = [sum(segs[:s]) for s in range(NSEG)]
    CTOT = sum(segs)
    # global chunk list: (seg, chunk-in-seg, out col offset, width)
    chunks = []
    for s in range(NSEG):
        for ci, (coff, ck) in enumerate(seg_cbs[s]):
            chunks.append((s, ci, seg_off[s] + coff, ck))
    NCH = len(chunks)

    nc = bacc.Bacc("TRN2", target_bir_lowering=False, debug=False,
                   num_devices=N_CORES)

    ck0 = chunks[0][3]
    wall = nc.dram_tensor("wall", [NSL * P, WSL], BF16, kind="ExternalInput")
    # x: chunk 0 tight ([P, KH*ck0], cols k*ck0 + c) so the start-window
    # transfer is minimal; chunks 1+ are [P, KH*CKS] blocks (cols
    # k*CKS + c, zero-padded past ck) so slot DMA is fully contiguous
    xc = nc.dram_tensor("xc", [P, KH * ck0 + (NCH - 1) * KH * CKS], BF16,
                        kind="ExternalInput")
    b1c = nc.dram_tensor("b1c", [P, NSEG * FL // P], FP32,
                         kind="ExternalInput")
    out = nc.dram_tensor("out", [H, CTOT], FP16, kind="ExternalOutput")

    out_v = out.rearrange("(t p) c -> p t c", p=P)   # [128, 8, CTOT]

    GELU = mybir.ActivationFunctionType.Gelu

    with TileContext(nc) as tc:
        with (
            tc.tile_pool(name="const", bufs=1) as constp,
            tc.tile_pool(name="xp", bufs=4) as xp,
            tc.tile_pool(name="wp", bufs=1) as wp,
            tc.tile_pool(name="hp", bufs=1) as hp,
            tc.tile_pool(name="op", bufs=4) as op,
            tc.tile_pool(name="ps1", bufs=2, space="PSUM") as ps1p,
            tc.tile_pool(name="psy", bufs=1, space="PSUM") as psyp,
        ):
            zt = constp.tile([P, 2 * P], BF16, tag="zt")
            nc.vector.memset(zt[:], 0.0)
            wups = psyp.tile([P, 2, 256], FP32, tag="warm")
            for i in range(N_WARMUP):
                nc.tensor.matmul(wups[:, i % 2, :128], zt[:, :P],
                                 zt[:, :128], start=True, stop=True)

            w_sb = [wp.tile([P, WSL], BF16, tag=f"w_{sl}", name=f"w_{sl}")
                    for sl in range(NSL)]

            x_t = []

            def load_x(g):
                t = xp.tile([P, KH, CKS], BF16, tag="x", name=f"x_{g}")
                base = KH * ck0 + (g - 1) * KH * CKS
                nc.sync.dma_start(
                    out=t[:], in_=xc[:, base:base + KH * CKS])
                x_t.append(t)

            # start window: tight x chunk0 + slab(0) w1 m0 interleaved,
            # b1, rest of slab0 w1, slab1 w1 (seg0's GEMM1 set), then x c1
            x0t = constp.tile([P, KH, ck0], BF16, tag="x0", name="x_0")
            x_t.append(x0t)
            nc.sync.dma_start(out=x0t[:, :2, :], in_=xc[:, :2 * ck0])
            nc.sync.dma_start(out=w_sb[0][:, :4 * P], in_=wall[:P, :4 * P])
            nc.sync.dma_start(out=x0t[:, 2:4, :],
                              in_=xc[:, 2 * ck0:4 * ck0])
            nc.sync.dma_start(out=w_sb[0][:, 4 * P:KH * P],
                              in_=wall[:P, 4 * P:KH * P])
            nc.sync.dma_start(out=x0t[:, 4:, :],
                              in_=xc[:, 4 * ck0:KH * ck0])
            b1_sb = constp.tile([P, NSEG * FL // P], FP32, tag="b1")
            nc.sync.dma_start(out=b1_sb[:], in_=b1c[:])
            for m in range(1, MF):
                nc.sync.dma_start(
                    out=w_sb[0][:, m * KH * P:(m + 1) * KH * P],
                    in_=wall[:P, m * KH * P:(m + 1) * KH * P])
            nc.sync.dma_start(out=w_sb[1][:, :WS1],
                              in_=wall[P:2 * P, :WS1])
            if NCH > 1:
                load_x(1)
            # seg0 w2, x c2, seg1 w1, x c3, seg1 w2, then remaining
            # segs' weights (x c4+ are emitted inline in the chunk loop)
            for sl in (0, 1):
                nc.sync.dma_start(out=w_sb[sl][:, WS1:],
                                  in_=wall[sl * P:(sl + 1) * P, WS1:])
            if NCH > 2:
                load_x(2)
            for sl in (2, 3):
                nc.sync.dma_start(out=w_sb[sl][:, :WS1],
                                  in_=wall[sl * P:(sl + 1) * P, :WS1])
            if NCH > 3:
                load_x(3)
            for sl in (2, 3):
                nc.sync.dma_start(out=w_sb[sl][:, WS1:],
                                  in_=wall[sl * P:(sl + 1) * P, WS1:])
            for s in (2, 3):
                for part in range(2):
                    for sl in (2 * s, 2 * s + 1):
                        r0, r1 = sl * P, (sl + 1) * P
                        if part == 0:
                            nc.sync.dma_start(out=w_sb[sl][:, :WS1],
                                              in_=wall[r0:r1, :WS1])
                        else:
                            nc.sync.dma_start(out=w_sb[sl][:, WS1:],
                                              in_=wall[r0:r1, WS1:])

            def w1sl(sl, m, k):
                return w_sb[sl][:, m * KH * P + k * P:m * KH * P + (k + 1) * P]

            def w2sl(sl, m, ht):
                base = WS1 + m * H + ht * P
                return w_sb[sl][:, base:base + P]

            for g, (s, ci, gcoff, ck) in enumerate(chunks):
                last_chunk = g == NCH - 1

                def ytile(q, half):
                    return psyp.tile([P, 512], FP32, tag=f"y{q}",
                                     name=f"y{q}_{g}_{half}")

                def o4tile(half):
                    return op.tile([P, 4, 512], FP16, tag="o4",
                                   name=f"o4_{g}_{half}")

                # GEMM1: h for the segment's 2 slabs staged in SBUF
                hL = hp.tile([P, NTHL, MF, 512], BF16, tag="h",
                             name=f"h_{g}")
                for th in range(NTHL):
                    sl = s * NTHL + th
                    for m in range(MF):
                        pt = ps1p.tile([P, 512], FP32, tag="ps1")
                        for k in range(KH):
                            nc.tensor.matmul(
                                pt[:, :ck],
                                w1sl(sl, m, k),
                                x_t[g][:, k, :ck],
                                start=(k == 0), stop=(k == KH - 1),
                            )
                        bidx = s * (FL // P) + th * MF + m
                        nc.scalar.activation(
                            hL[:, th, m, :ck], pt[:, :ck], GELU,
                            bias=b1_sb[:, bidx:bidx + 1],
                        )

                # prefetch x for chunk g+4 (chunks 0-3 preloaded; slot
                # (g+4)%4 is chunk g's own, whose readers — this chunk's
                # GEMM1, just emitted — retire before the DMA fires)
                if g + 4 < NCH:
                    load_x(g + 4)

                for half in range(2):
                    y_q = [ytile(q, half) for q in range(4)]
                    o4 = o4tile(half)

                    def evict(q):
                        nc.vector.tensor_copy(
                            o4[:, q, :ck], y_q[q][:, :ck])

                    if last_chunk and half == 1:
                        for q in range(4):
                            ht = 4 * half + q
                            for th in range(NTHL):
                                sl = s * NTHL + th
                                for m in range(MF):
                                    nc.tensor.matmul(
                                        y_q[q][:, :ck],
                                        w2sl(sl, m, ht),
                                        hL[:, th, m, :ck],
                                        start=(th == 0 and m == 0),
                                        stop=(th == NTHL - 1 and m == MF - 1),
                                    )
                            evict(q)
                            nc.sync.dma_start(
                                out=out_v[:, ht:ht + 1, gcoff:gcoff + ck],
                                in_=o4[:, q:q + 1, :ck])
                        continue
                    else:
                        for th in range(NTHL):
                            sl = s * NTHL + th
                            for m in range(MF):
                                for q in range(4):
                                    nc.tensor.matmul(
                                        y_q[q][:, :ck],
                                        w2sl(sl, m, 4 * half + q),
                                        hL[:, th, m, :ck],
                                        start=(th == 0 and m == 0),
                                        stop=(th == NTHL - 1 and m == MF - 1),
                                    )
                        for q in range(4):
                            evict(q)
                    # mid-kernel stores ride the Act HWDGE queue: slow
                    # (~35GB/s) but idle, so they never queue behind the
                    # 25MB weight/x backlog on the SP queue — which stalls
                    # the o4-slot WAR chain into the PE (22us when stores
                    # rode SP; 4.6us even alternating h0 stores onto SP).
                    # The last chunk's stores use the by-then-empty SP
                    # queue for a fast tail.
                    eng = nc.sync if last_chunk else nc.scalar
                    eng.dma_start(
                        out=out_v[:, 4 * half:4 * half + 4, gcoff:gcoff + ck],
                        in_=o4[:, :, :ck])

    nc.compile()
    return nc


def _route(x: np.ndarray, router_w: np.ndarray):
    logits = x @ router_w.T                                   # [T, E]
    top_i = np.argsort(-logits, axis=1, kind="stable")[:, :TOPK]
    top_v = np.take_along_axis(logits, top_i, axis=1)
    mx = top_v.max(axis=1, keepdims=True)
    ex = np.exp(top_v - mx)
    rw = ex / ex.sum(axis=1, keepdims=True)
    return top_i, rw.astype(np.float32)


def _swizzle_wall_q(w1, w2, experts, j):
    """Core (quad, j)'s [NSL*P, WSL] slab matrix: per expert e (segment
    order), 2 slabs of 512 F rows from e's F-slice [j*FL,(j+1)*FL)."""
    rows = []
    for e in experts:
        w2t = w2[e].T                                   # [F, H]
        for th in range(NTHL):
            f0 = j * FL + th * 512
            a = (w1[e][f0:f0 + 512]                     # [512, H]
                 .reshape(MF, P, KH, P).transpose(3, 0, 2, 1)
                 .reshape(P, MF * KH * P))
            b = (w2t[f0:f0 + 512]                       # [512, H]
                 .reshape(MF, P, H).transpose(1, 0, 2)
                 .reshape(P, MF * H))
            rows.append(np.concatenate([a, b], axis=1))
    return np.ascontiguousarray(
        np.concatenate(rows, axis=0)).astype(ml_dtypes.bfloat16)


def kernel(hidden_states, router_w, w1, b1, w2, b2):
    hidden_states = np.ascontiguousarray(np.asarray(hidden_states, np.float32))
    router_w = np.ascontiguousarray(np.asarray(router_w, np.float32))
    w1 = np.asarray(w1, np.float32)
    b1 = np.asarray(b1, np.float32)
    w2 = np.asarray(w2, np.float32)
    b2 = np.asarray(b2, np.float32)

    B, S, _ = hidden_states.shape
    T = B * S
    x = hidden_states.reshape(T, H)

    top_i, rw = _route(x, router_w)

    sel_idx = []
    sel_gate = []
    counts = np.zeros(E, np.int64)
    for e in range(E):
        mask = top_i == e
        rows = np.nonzero(mask.any(axis=1))[0]
        g = rw[rows[:, None], np.argmax(mask[rows], axis=1)[:, None]][:, 0]
        sel_idx.append(rows)
        sel_gate.append(g.astype(np.float32))
        counts[e] = len(rows)

    # interleave sorted experts into 2 quads so rank-r loads match
    order = np.argsort(-counts, kind="stable")
    quads = [order[0::2], order[1::2]]
    segs = tuple(
        max(8, -(-int(max(counts[quads[0][r]], counts[quads[1][r]])) // 8) * 8)
        for r in range(NSEG))
    seg_off = [sum(segs[:s]) for s in range(NSEG)]

    if segs not in _cache:
        _cache[segs] = _build(segs)
    nc = _cache[segs]

    in_maps = [None] * N_CORES
    for q, experts in enumerate(quads):
        # xc shared by the quad's 4 cores: chunk 0 tight [P, KH*ck0],
        # later chunks zero-padded [P, KH*CKS] blocks, col = k*W + c
        blocks = []
        first = True
        for r in range(NSEG):
            e = experts[r]
            Ss = segs[r]
            xpad = np.zeros((Ss, H), np.float32)
            n_e = counts[e]
            xpad[:n_e] = x[sel_idx[e]]
            for coff, ck in _chunks(Ss):
                W = ck if first else CKS
                first = False
                blk = np.zeros((P, KH, W), np.float32)
                blk[:, :, :ck] = (xpad[coff:coff + ck]
                                  .reshape(ck, KH, P).transpose(2, 1, 0))
                blocks.append(blk.reshape(P, KH * W))
        xq = np.ascontiguousarray(
            np.concatenate(blocks, axis=1)).astype(ml_dtypes.bfloat16)

        for j in range(NSPLIT):
            b1j = np.concatenate(
                [b1[experts[r]][j * FL:(j + 1) * FL].reshape(FL // P, P).T
                 for r in range(NSEG)], axis=1)
            in_maps[q * NSPLIT + j] = {
                "wall": _swizzle_wall_q(w1, w2, experts, j),
                "xc": xq,
                "b1c": np.ascontiguousarray(b1j),
            }

    global LAST_EXEC_TIME_NS
    LAST_EXEC_TIME_NS = 0
    res = run_bass_kernel_spmd(nc, in_maps, list(range(N_CORES)), trace=TRACE)
    if res.exec_time_ns:
        LAST_EXEC_TIME_NS = res.exec_time_ns

    out = np.zeros((T, H), np.float32)
    for q, experts in enumerate(quads):
        for r in range(NSEG):
            e = experts[r]
            rows, g = sel_idx[e], sel_gate[e]
            if not len(rows):
                continue
            o = seg_off[r]
            y = sum(res.results[q * NSPLIT + j]["out"][:, o:o + len(rows)]
                    .astype(np.float32) for j in range(NSPLIT))
            out[rows] += g[:, None] * (y.T + b2[e][None, :])
    return out.reshape(B, S, H)
